# revision 3
# baseline (speedup 1.0000x reference)
"""CenterLoss Trainium2 kernel (v2: stock indirect DMA gather).

loss = (sum_i clamp(||x_i - centers[labels_i]||^2, 1e-12, 1e12)
        + BS*(C_OUT-1)*1e-12) / BS

Masking the full [BS, C_OUT] distance matrix keeps exactly one distance
per row; the other BS*(C_OUT-1) clamped zeros are a host-side constant.

Data-parallel over batch across 8 NeuronCores, centers replicated.
Each core gathers its 1024 label rows with ONE stock indirect DMA
(InstDMACopy + dynamic_ap_info on the Pool SWDGE queue).  Unlike the
dma_gather ucode path this needs NO Q7 library swap (~10us saved), takes
int32 indices (no int16 pair/parity trick -> half the gather bytes and
half the vector math), and the resident DGE backend generates
descriptors far faster than the mlp gather kernel's ~8.5ns/desc.

x and labels ride ONE hardware DMA: host packs [128, 520] f32 rows
(512 x floats + 8 int32 labels bit-cast) so the whole input lands with
128 contiguous 2080B descriptors.  Row r of the core's shard maps to
(partition r//8, slot r%8); the gather output follows the same order
because the offset AP is the packed label slice.

Final reduction on device: DVE row-sums + clamp -> [128, 8] -> [128, 1],
then a ones-vector fp32 matmul folds partitions -> [1, 1] scalar, so the
output DMA is a single 4-byte descriptor.  Host sums 8 core scalars in
f64 and adds the clamp constant.

Host-side input prep is limited to sharding/replication, dtype narrowing
(int64 -> int32), and layout packing; all arithmetic runs on device.
"""

import numpy as np

BS, C_OUT, D = 8192, 50000, 64
N_CORES = 8
ROWS = BS // N_CORES  # rows per core
P = 128  # SBUF partitions
RPP = ROWS // P  # rows per partition = 8
XF = RPP * D  # x floats per partition = 512
XLW = XF + RPP  # packed row width = 520 (512 x + 8 label words)
CLAMP_MIN, CLAMP_MAX = 1e-12, 1e12

_CACHE = {}


def _build_program():
    import concourse.bacc as bacc
    import concourse.bass as bass
    import concourse.mybir as mybir

    nc = bacc.Bacc(
        "TRN2", target_bir_lowering=False, debug=False, num_devices=N_CORES
    )

    f32 = mybir.dt.float32
    i32 = mybir.dt.int32

    xlab_d = nc.dram_tensor("xlab", [P, XLW], f32, kind="ExternalInput")
    cen_d = nc.dram_tensor("centers", [C_OUT, D], f32, kind="ExternalInput")
    out_d = nc.dram_tensor("out", [1, 1], f32, kind="ExternalOutput")

    from contextlib import ExitStack
    with ExitStack() as ctx:
        xlab_t = ctx.enter_context(nc.sbuf_tensor("xlab_t", [P, XLW], f32))
        g_t = ctx.enter_context(nc.sbuf_tensor("g_t", [P, XF], f32))
        d_t = ctx.enter_context(nc.sbuf_tensor("d_t", [P, XF], f32))
        s_t = ctx.enter_context(nc.sbuf_tensor("s_t", [P, RPP], f32))
        cl_t = ctx.enter_context(nc.sbuf_tensor("cl_t", [P, RPP], f32))
        r_t = ctx.enter_context(nc.sbuf_tensor("r_t", [P, 1], f32))
        ones_t = ctx.enter_context(nc.sbuf_tensor("ones_t", [P, 1], f32))
        res_t = ctx.enter_context(nc.sbuf_tensor("res_t", [1, 1], f32))
        ps_t = ctx.enter_context(nc.psum_tensor("ps_t", [1, 1], f32))
        s_in = ctx.enter_context(nc.semaphore("s_in"))
        s_g = ctx.enter_context(nc.semaphore("s_g"))
        s_v = ctx.enter_context(nc.semaphore("s_v"))
        s_mm = ctx.enter_context(nc.semaphore("s_mm"))
        s_res = ctx.enter_context(nc.semaphore("s_res"))
        s_out = ctx.enter_context(nc.semaphore("s_out"))
        block = ctx.enter_context(nc.Block())

        @block.sync
        def _(sync: bass.BassEngine):
            # one DMA carries x AND labels: 128 contiguous 2080B descriptors
            sync.dma_start(out=xlab_t[:], in_=xlab_d[:]).then_inc(s_in, 16)
            # scalar writeback; no completion wait -- NEFF epilogue drains
            sync.wait_ge(s_res, 1)
            sync.dma_start(out=out_d[:], in_=res_t[:]).then_inc(s_out, 16)

        @block.gpsimd
        def _(gpsimd: bass.BassGpSimd):
            gpsimd.memset(ones_t[:], 1.0)
            gpsimd.wait_ge(s_in, 16)
            # stock indirect gather: g[p, c, :] = centers[labels[p, c], :]
            gpsimd.indirect_dma_start(
                out=g_t[:],
                out_offset=None,
                in_=cen_d[:],
                in_offset=bass.IndirectOffsetOnAxis(
                    ap=xlab_t[:, XF:XLW].bitcast(i32),
                    axis=0,
                ),
            ).then_inc(s_g, 16)

        @block.vector
        def _(vector: bass.BassEngine):
            # DVE has no same-engine interlock: s_v counts completions
            vector.wait_ge(s_g, 16)
            vector.tensor_tensor(
                out=d_t[:],
                in0=xlab_t[:, 0:XF],
                in1=g_t[:],
                op=mybir.AluOpType.subtract,
            ).then_inc(s_v, 1)
            vector.wait_ge(s_v, 1)
            vector.tensor_tensor(
                out=d_t[:], in0=d_t[:], in1=d_t[:], op=mybir.AluOpType.mult
            ).then_inc(s_v, 1)
            vector.wait_ge(s_v, 2)
            vector.reduce_sum(
                out=s_t[:],
                in_=d_t[:].rearrange("p (n m) -> p n m", m=D),
                axis=mybir.AxisListType.X,
            ).then_inc(s_v, 1)
            vector.wait_ge(s_v, 3)
            vector.tensor_scalar(
                out=cl_t[:],
                in0=s_t[:],
                scalar1=CLAMP_MIN,
                scalar2=CLAMP_MAX,
                op0=mybir.AluOpType.max,
                op1=mybir.AluOpType.min,
            ).then_inc(s_v, 1)
            vector.wait_ge(s_v, 4)
            vector.reduce_sum(
                out=r_t[:],
                in_=cl_t[:],
                axis=mybir.AxisListType.X,
            ).then_inc(s_v, 1)
            # PSUM -> SBUF for the writeback
            vector.wait_ge(s_mm, 1)
            vector.tensor_copy(out=res_t[:], in_=ps_t[:]).then_inc(s_res, 1)

        @block.tensor
        def _(tensor: bass.BassEngine):
            # partition fold: ones[128,1].T @ r[128,1] -> [1,1]
            tensor.wait_ge(s_v, 5)
            tensor.matmul(
                out=ps_t[:],
                lhsT=ones_t[:],
                rhs=r_t[:],
                start=True,
                stop=True,
            ).then_inc(s_mm, 1)

    nc.compile()
    return nc


def _get_program():
    if "nc" not in _CACHE:
        _CACHE["nc"] = _build_program()
    return _CACHE["nc"]


def kernel(x, labels, centers, trace=False):
    from concourse.bass_utils import run_bass_kernel_spmd

    nc = _get_program()

    x = np.asarray(x, dtype=np.float32)
    labels_i32 = np.asarray(labels, dtype=np.int32)
    centers = np.ascontiguousarray(np.asarray(centers, dtype=np.float32))

    in_maps = []
    for i in range(N_CORES):
        x_c = x[i * ROWS : (i + 1) * ROWS].reshape(P, XF)
        lab_c = (
            labels_i32[i * ROWS : (i + 1) * ROWS].reshape(P, RPP).view(np.float32)
        )
        xlab = np.ascontiguousarray(
            np.concatenate([x_c, lab_c], axis=1, dtype=np.float32)
        )
        in_maps.append({"xlab": xlab, "centers": centers})

    res = run_bass_kernel_spmd(
        nc, in_maps, core_ids=list(range(N_CORES)), trace=trace
    )

    total = np.float64(0.0)
    for r in res.results:
        total += np.float64(r["out"][0, 0])
    # masked-out entries: BS*(C_OUT-1) zeros, each clamped to 1e-12
    total += np.float64(BS) * np.float64(C_OUT - 1) * 1e-12
    loss = np.float32(total / BS)

    if trace:
        _CACHE["last_exec_time_ns"] = res.exec_time_ns
        _CACHE["last_results"] = res
    return np.array(loss, dtype=np.float32)
